# revision 5
# baseline (speedup 1.0000x reference)
"""CrossModalFeatureInteraction kernel for Trainium2 (Bass/Tile), 8 NeuronCores.

Computation (per pixel, per batch):
    combined = concat([vis, ir], channel)              # [512]
    x        = relu(W1 @ combined + b1)                # [32]
    residual = W2 @ x + b2                             # [256]
    out      = vis + ir + residual                     # [256]

Sharding: data-parallel over batch. B=16 -> 2 images per core on 8 cores.
Weights are tiny and replicated. Each core streams its 2 images through
SBUF in pixel supertiles; 1x1 convs are matmuls with channels as the
contraction dim and pixels as the moving free dim.

Matmuls run in float32r (full-rate PE mode for fp32 data). The bypass
path (vis + ir) reads the same SBUF tiles bitcast back to plain f32, so
it is exact regardless of the matmul mode.
"""

import numpy as np

import concourse.bass as bass
import concourse.mybir as mybir
from concourse import bacc, bass_utils
from concourse.tile import TileContext

# Problem shape (hardcoded per contract)
B, C, H, W = 16, 256, 64, 64
HID = 32
HWPIX = H * W          # 4096 pixels per image
N_CORES = 8
B_PER_CORE = B // N_CORES  # 2

NBIG = 2048            # pixels per DMA supertile (1 MiB per [128, NBIG] f32 tile)
NT = 512               # matmul moving free dim (one PSUM bank of fp32)
KO = 4                 # 512 combined channels / 128 partitions
CCH = 2                # 256 output channels / 128 partitions

F32 = mybir.dt.float32
F32R = mybir.dt.float32r

_cache = {}


def _build(mm1: str, mm2: str) -> bass.Bass:
    """mm1/mm2: dtype mode for the first/second matmul: 'f32r' or 'f32'."""
    d1 = F32R if mm1 == "f32r" else F32
    d2 = F32R if mm2 == "f32r" else F32

    nc = bacc.Bacc(
        "TRN2", target_bir_lowering=False, debug=False, num_devices=N_CORES
    )
    vis = nc.dram_tensor("vis", [B_PER_CORE, C, HWPIX], d1, kind="ExternalInput")
    ir = nc.dram_tensor("ir", [B_PER_CORE, C, HWPIX], d1, kind="ExternalInput")
    w1t = nc.dram_tensor("w1t", [KO, 128, HID], d1, kind="ExternalInput")
    b1 = nc.dram_tensor("b1", [HID, 1], F32, kind="ExternalInput")
    w2t = nc.dram_tensor("w2t", [HID, C], d2, kind="ExternalInput")
    b2 = nc.dram_tensor("b2", [128, CCH], F32, kind="ExternalInput")
    out = nc.dram_tensor("out", [B_PER_CORE, C, HWPIX], F32, kind="ExternalOutput")

    with TileContext(nc) as tc:
        with (
            tc.tile_pool(name="consts", bufs=1) as cpool,
            tc.tile_pool(name="inbuf", bufs=2) as inpool,
            tc.tile_pool(name="work", bufs=3) as wpool,
            tc.tile_pool(name="outbuf", bufs=2) as opool,
            tc.tile_pool(name="ps1", bufs=2, space="PSUM") as ps1pool,
            tc.tile_pool(name="ps2", bufs=4, space="PSUM") as ps2pool,
        ):
            w1t_sb = cpool.tile([128, KO, HID], d1, tag="w1t")
            nc.sync.dma_start(w1t_sb, w1t[:, :, :].rearrange("ko p m -> p ko m"))
            w2t_sb = cpool.tile([HID, C], d2, tag="w2t")
            nc.sync.dma_start(w2t_sb, w2t[:, :])
            b1_sb = cpool.tile([HID, 1], F32, tag="b1")
            nc.sync.dma_start(b1_sb, b1[:, :])
            b2_sb = cpool.tile([128, CCH], F32, tag="b2")
            nc.sync.dma_start(b2_sb, b2[:, :])

            for b in range(B_PER_CORE):
                for j in range(HWPIX // NBIG):
                    jsl = slice(j * NBIG, (j + 1) * NBIG)
                    ins = {}
                    for nm, dram in (("v", vis), ("i", ir)):
                        for c in range(CCH):
                            t = inpool.tile(
                                [128, NBIG], d1, tag=f"in_{nm}{c}", name=f"in_{nm}{c}"
                            )
                            nc.sync.dma_start(t, dram[b, c * 128 : (c + 1) * 128, jsl])
                            ins[(nm, c)] = t
                    outs = [
                        opool.tile([128, NBIG], F32, tag=f"out{c}", name=f"outt{c}")
                        for c in range(CCH)
                    ]
                    for js in range(NBIG // NT):
                        sl = slice(js * NT, (js + 1) * NT)
                        ps1 = ps1pool.tile([HID, NT], F32, tag="ps1", name="ps1")
                        rhs_order = [("v", 0), ("v", 1), ("i", 0), ("i", 1)]
                        for ko, key in enumerate(rhs_order):
                            nc.tensor.matmul(
                                ps1,
                                lhsT=w1t_sb[:, ko],
                                rhs=ins[key][:, sl],
                                start=(ko == 0),
                                stop=(ko == KO - 1),
                            )
                        x_t = wpool.tile([HID, NT], d2, tag="x", name="x_t")
                        nc.scalar.activation(
                            x_t, ps1, mybir.ActivationFunctionType.Relu,
                            bias=b1_sb[:, 0:1],
                        )
                        for c in range(CCH):
                            ps2 = ps2pool.tile([128, NT], F32, tag="ps2", name="ps2")
                            nc.tensor.matmul(
                                ps2,
                                lhsT=w2t_sb[:, c * 128 : (c + 1) * 128],
                                rhs=x_t,
                                start=True,
                                stop=True,
                            )
                            s_t = wpool.tile([128, NT], F32, tag="s", name="s_t")
                            nc.vector.tensor_add(
                                s_t,
                                ins[("v", c)][:, sl].bitcast(F32),
                                ins[("i", c)][:, sl].bitcast(F32),
                            )
                            r_t = wpool.tile([128, NT], F32, tag="r", name="r_t")
                            nc.scalar.activation(
                                r_t, ps2, mybir.ActivationFunctionType.Identity,
                                bias=b2_sb[:, c : c + 1],
                            )
                            nc.vector.tensor_add(outs[c][:, sl], s_t, r_t)
                    for c in range(CCH):
                        nc.sync.dma_start(
                            out[b, c * 128 : (c + 1) * 128, jsl], outs[c]
                        )
    nc.compile()
    return nc


def _get_nc(mm1: str, mm2: str) -> bass.Bass:
    key = ("nc", mm1, mm2)
    if key not in _cache:
        _cache[key] = _build(mm1, mm2)
    return _cache[key]


def kernel(
    visible_features: np.ndarray,
    infrared_features: np.ndarray,
    W1: np.ndarray,
    b1: np.ndarray,
    W2: np.ndarray,
    b2: np.ndarray,
    _mm1: str = "f32r",
    _mm2: str = "f32r",
    _trace: bool = False,
) -> np.ndarray:
    nc = _get_nc(_mm1, _mm2)

    vis = np.ascontiguousarray(visible_features, dtype=np.float32).reshape(B, C, HWPIX)
    ir = np.ascontiguousarray(infrared_features, dtype=np.float32).reshape(B, C, HWPIX)
    w1t = np.ascontiguousarray(W1.astype(np.float32).T.reshape(KO, 128, HID))
    b1r = np.ascontiguousarray(b1.astype(np.float32).reshape(HID, 1))
    w2t = np.ascontiguousarray(W2.astype(np.float32).T)  # [HID, C]
    b2r = np.ascontiguousarray(b2.astype(np.float32).reshape(CCH, 128).T)  # [128, CCH]

    in_maps = []
    for core in range(N_CORES):
        bsl = slice(core * B_PER_CORE, (core + 1) * B_PER_CORE)
        in_maps.append(
            {
                "vis": vis[bsl],
                "ir": ir[bsl],
                "w1t": w1t,
                "b1": b1r,
                "w2t": w2t,
                "b2": b2r,
            }
        )

    res = bass_utils.run_bass_kernel_spmd(
        nc, in_maps, core_ids=list(range(N_CORES)), trace=_trace
    )
    if _trace:
        kernel.last_results = res
    outs = [r["out"] for r in res.results]
    return np.concatenate(outs, axis=0).reshape(B, C, H, W)
